# revision 29
# baseline (speedup 1.0000x reference)
"""Trainium2 Bass kernel for nn_AttentionBlock (LN -> single-head attention -> residual).

Sharding: pure data parallelism — batch B=8, one batch element per NeuronCore.
Per core (N=4096 tokens, C=256 channels):
  LN (token-partition layout) -> PE transpose to hT [C, N] -> QT/KT/V projections
  -> per-1024-query-chunk: scoresT (keys on partitions, fp8 DoubleRow K=256)
     -> exp (ACT, fp8 out, exp(s - ln4) to fit e4m3 range) -> PV + rowsum
     matmuls (fp8 DoubleRow over key pairs) -> normalize
  -> output projection + residual (final phase).
Projections/transpose in bf16; attention matmuls in fp8e4m3 with DoubleRow
(2 fp8/cell, K=256 per matmul); all accumulation in fp32 PSUM except the
single-shot scores matmul (bf16 PSUM). The exp/rowsum scale factor 1/4
cancels in the softmax normalization.
gamma/beta/attention-scale/biases are folded into the weights on the host:
  wq' = gamma*wq/sqrt(C), bq' = (beta@wq + bq)/sqrt(C), wk' = gamma*wk,
  bk' = beta@wk + bk, wv' = gamma*wv, beff = bo + (beta@wv + bv)@wo.
"""

import math
import os
import sys

for _p in ("/opt/trn_rl_repo", "/opt/pypackages"):
    if os.path.isdir(_p) and _p not in sys.path:
        sys.path.append(_p)

import ml_dtypes
import numpy as np

B, H, W, C = 8, 64, 64, 256
N = H * W          # 4096 tokens per batch element
P = 128            # partitions
NT = N // P        # 32 token tiles / key blocks
KC = C // P        # 2 channel sub-tiles
CHUNK = 1024       # query-chunk size
NCH = N // CHUNK   # 4 chunks
HB = 512           # psum half-bank split for fp32 accumulators
LN_EPS = 1e-3
LN4 = math.log(4.0)

_CACHE = {}


def _build_nc():
    import concourse.mybir as mybir
    import concourse.tile as tile
    from concourse.bacc import Bacc
    from concourse.masks import make_identity

    f32 = mybir.dt.float32
    bf16 = mybir.dt.bfloat16
    f8 = mybir.dt.float8e4
    AF = mybir.ActivationFunctionType
    Alu = mybir.AluOpType
    DR = mybir.MatmulPerfMode.DoubleRow

    nc = Bacc()

    x_d = nc.declare_dram_parameter("x", [N, C], f32, isOutput=False)
    wq_d = nc.declare_dram_parameter("wq", [P, KC, C], bf16, isOutput=False)
    wk_d = nc.declare_dram_parameter("wk", [P, KC, C], bf16, isOutput=False)
    wv_d = nc.declare_dram_parameter("wv", [P, KC, C], bf16, isOutput=False)
    wo_d = nc.declare_dram_parameter("wo", [P, KC, C], bf16, isOutput=False)
    bq_d = nc.declare_dram_parameter("bq", [P, KC], f32, isOutput=False)
    bk_d = nc.declare_dram_parameter("bk", [P, KC], f32, isOutput=False)
    beff_d = nc.declare_dram_parameter("beff", [P, C], f32, isOutput=False)
    out_d = nc.declare_dram_parameter("out", [N, C], f32, isOutput=True)

    x_r = x_d[:].rearrange("(t p) c -> p t c", p=P)
    out_r = out_d[:].rearrange("(t p) c -> p t c", p=P)

    with tile.TileContext(nc) as tc:
        with (
            tc.tile_pool(name="consts", bufs=1) as consts,
            tc.tile_pool(name="big", bufs=1) as big,
            tc.tile_pool(name="lnp", bufs=4) as lnp,
            tc.tile_pool(name="expp", bufs=20) as expp,
            tc.tile_pool(name="rsbp", bufs=2) as rsbp,
            tc.tile_pool(name="outp", bufs=2) as outp,
            tc.tile_pool(name="ps_s", bufs=2, space="PSUM") as ps_s,
            tc.tile_pool(name="ps_pv", bufs=1, space="PSUM") as ps_pv,
            tc.tile_pool(name="ps_rs", bufs=1, space="PSUM") as ps_rs,
        ):
            # ---- constants / weights ----
            wq_sb = consts.tile([P, KC, C], bf16, tag="wq")
            wk_sb = consts.tile([P, KC, C], bf16, tag="wk")
            wv_sb = consts.tile([P, KC, C], bf16, tag="wv")
            wo_sb = consts.tile([P, KC, C], bf16, tag="wo")
            bq_sb = consts.tile([P, KC], f32, tag="bq")
            bk_sb = consts.tile([P, KC], f32, tag="bk")
            beff_sb = consts.tile([P, C], f32, tag="beff")
            nc.sync.dma_start(out=wq_sb, in_=wq_d[:])
            nc.sync.dma_start(out=wk_sb, in_=wk_d[:])
            nc.sync.dma_start(out=wv_sb, in_=wv_d[:])
            nc.sync.dma_start(out=wo_sb, in_=wo_d[:])
            nc.sync.dma_start(out=bq_sb, in_=bq_d[:])
            nc.sync.dma_start(out=bk_sb, in_=bk_d[:])
            nc.sync.dma_start(out=beff_sb, in_=beff_d[:])

            ident = consts.tile([P, P], bf16, tag="ident")
            make_identity(nc, ident)
            ones_sb = consts.tile([P, KC, P], f8, tag="ones")
            nc.vector.memset(ones_sb, 1.0)
            eps_sb = consts.tile([P, 1], f32, tag="eps")
            nc.vector.memset(eps_sb, LN_EPS)
            nln4_sb = consts.tile([P, 1], f32, tag="nln4")
            nc.vector.memset(nln4_sb, -LN4)

            # ---- load x (quartered for pipelining) ----
            NTQ = NT // NCH  # token tiles per quarter (8)
            x_q = [
                big.tile([P, NTQ, C], f32, tag=f"x{q}", name=f"x_q{q}")
                for q in range(NCH)
            ]
            for q in range(NCH):
                nc.sync.dma_start(
                    out=x_q[q], in_=x_r[:, q * NTQ : (q + 1) * NTQ, :]
                )

            # ---- LayerNorm (no affine; folded into weights) ----
            # stats on DVE, normalize applied on ACT: (x - mean)*rstd =
            # Identity(x*rstd + (-mean*rstd))
            xn = big.tile([P, NT, C], bf16, tag="xn")
            for t in range(NT):
                x_t = x_q[t // NTQ][:, t % NTQ, :]
                stats = lnp.tile([P, 6], f32, tag="stats")
                nc.vector.bn_stats(out=stats, in_=x_t)
                mv = lnp.tile([P, 2], f32, tag="mv")
                nc.vector.bn_aggr(out=mv, in_=stats)
                std = lnp.tile([P, 1], f32, tag="std")
                nc.scalar.activation(
                    out=std, in_=mv[:, 1:2], func=AF.Sqrt, bias=eps_sb
                )
                rstd = lnp.tile([P, 1], f32, tag="rstd")
                nc.vector.reciprocal(out=rstd, in_=std)
                nmr = lnp.tile([P, 1], f32, tag="nmr")
                nc.vector.tensor_scalar(
                    out=nmr,
                    in0=mv[:, 0:1],
                    scalar1=rstd,
                    scalar2=-1.0,
                    op0=Alu.mult,
                    op1=Alu.mult,
                )
                nc.scalar.activation(
                    out=xn[:, t, :],
                    in_=x_t,
                    func=AF.Identity,
                    bias=nmr,
                    scale=rstd,
                )

            # ---- transpose xn -> hT [C(part), N(free)], quartered ----
            hT_q = [
                big.tile([P, KC, CHUNK], bf16, tag=f"hT{q}", name=f"hT_q{q}")
                for q in range(NCH)
            ]
            for cj in range(KC):
                for tq in range(NT // 4):
                    t_ps = ps_s.tile([P, 512], bf16, tag="s")
                    for j in range(4):
                        ti = tq * 4 + j
                        nc.tensor.transpose(
                            t_ps[:, j * P : (j + 1) * P],
                            xn[:, ti, cj * P : (cj + 1) * P],
                            ident,
                        )
                    q = tq // 2
                    off = (tq % 2) * 512
                    nc.scalar.copy(
                        out=hT_q[q][:, cj, off : off + 512], in_=t_ps
                    )

            # ---- Q/K projections -> qT, kT [Cqk(part), N(free)], fp8 ----
            qT = big.tile([P, KC, N], f8, tag="qT")
            kT = big.tile([P, KC, N], f8, tag="kT")
            for w_sb, b_sb, dst in ((wq_sb, bq_sb, qT), (wk_sb, bk_sb, kT)):
                for c2 in range(KC):
                    for nch in range(NCH):
                        ps = ps_s.tile([P, CHUNK], f32, tag="s")
                        for hb in range(CHUNK // HB):
                            off = hb * HB
                            for kc in range(KC):
                                nc.tensor.matmul(
                                    ps[:, hb * HB : (hb + 1) * HB],
                                    lhsT=w_sb[:, kc, c2 * P : (c2 + 1) * P],
                                    rhs=hT_q[nch][:, kc, off : off + HB],
                                    start=(kc == 0),
                                    stop=(kc == KC - 1),
                                )
                        nc.scalar.activation(
                            out=dst[:, c2, nch * CHUNK : (nch + 1) * CHUNK],
                            in_=ps,
                            func=AF.Identity,
                            bias=b_sb[:, c2 : c2 + 1],
                        )

            # ---- V projection -> v [m(part), Cv(free)], fp8 ----
            v_sb = big.tile([P, NT, C], f8, tag="v")
            for mb in range(NT):
                ps = ps_s.tile([P, C], f32, tag="s")
                q, off = mb // NTQ, (mb % NTQ) * P
                for kc in range(KC):
                    nc.tensor.matmul(
                        ps,
                        lhsT=hT_q[q][:, kc, off : off + P],
                        rhs=wv_sb[:, kc, :],
                        start=(kc == 0),
                        stop=(kc == KC - 1),
                    )
                nc.scalar.copy(out=v_sb[:, mb, :], in_=ps)

            # ---- attention, one 1024-query chunk at a time ----
            # normalized attention output (transposed layout) for ALL chunks;
            # reuses xn's SBUF slot (xn's last reader is the transpose phase)
            attnT = big.tile([P, KC, N], bf16, tag="xn")
            NPAIR = NT // 2
            for ch in range(NCH):
                q_sl = slice(ch * CHUNK, (ch + 1) * CHUNK)
                rs_ps = ps_rs.tile([P, CHUNK], f32, tag="rs")
                es = []
                # phase a: scores (fp8 DR, K=256 one-shot) -> exp -> fp8 pairs,
                # with the ones-matmul row-sum riding along
                for pr in range(NPAIR):
                    e = expp.tile([P, 2, CHUNK], f8, tag="e")
                    es.append(e)
                    for j in range(2):
                        mb = 2 * pr + j
                        s_ps = ps_s.tile([P, CHUNK], f32, tag="s")
                        for hb in range(CHUNK // HB):
                            nc.tensor.matmul(
                                s_ps[:, hb * HB : (hb + 1) * HB],
                                lhsT=kT[:, 0:KC, mb * P : (mb + 1) * P],
                                rhs=qT[:, 0:KC, ch * CHUNK + hb * HB : ch * CHUNK + (hb + 1) * HB],
                                start=True,
                                stop=True,
                                perf_mode=DR,
                            )
                        # exp(s - ln4): scale by 1/4 so e4m3 max (448) is safe;
                        # the factor cancels between PV and the row-sum.
                        nc.scalar.activation(
                            out=e[:, j, :], in_=s_ps, func=AF.Exp, bias=nln4_sb
                        )
                    for hb in range(CHUNK // HB):
                        nc.tensor.matmul(
                            rs_ps[:, hb * HB : (hb + 1) * HB],
                            lhsT=ones_sb,
                            rhs=e[:, 0:2, hb * HB : (hb + 1) * HB],
                            start=(pr == 0),
                            stop=(pr == NPAIR - 1),
                            perf_mode=DR,
                            skip_group_check=True,
                        )
                # reciprocal of row-sums (replicated across partitions)
                rs_sb = rsbp.tile([P, CHUNK], f32, tag="rssb")
                nc.vector.reciprocal(out=rs_sb, in_=rs_ps)
                # phase b: PV (fp8 DR over key pairs), one c-tile at a time
                for c2 in range(KC):
                    pv = ps_pv.tile([P, CHUNK], f32, tag="pv")
                    for pr in range(NPAIR):
                        for hb in range(CHUNK // HB):
                            nc.tensor.matmul(
                                pv[:, hb * HB : (hb + 1) * HB],
                                lhsT=v_sb[:, 2 * pr : 2 * pr + 2, c2 * P : (c2 + 1) * P],
                                rhs=es[pr][:, 0:2, hb * HB : (hb + 1) * HB],
                                start=(pr == 0),
                                stop=(pr == NPAIR - 1),
                                perf_mode=DR,
                                skip_group_check=True,
                            )
                    nc.vector.tensor_mul(
                        out=attnT[:, c2, q_sl], in0=pv, in1=rs_sb
                    )

                # ---- output projection + residual + bias for this chunk ----
                NS = CHUNK // P
                ot = outp.tile([P, NS, C], f32, tag="out")
                for ns in range(NS):
                    o_ps = ps_pv.tile([P, C], f32, tag="pv")
                    for kc in range(KC):
                        nc.tensor.matmul(
                            o_ps,
                            lhsT=attnT[
                                :, kc, ch * CHUNK + ns * P : ch * CHUNK + (ns + 1) * P
                            ],
                            rhs=wo_sb[:, kc, :],
                            start=(kc == 0),
                            stop=(kc == KC - 1),
                        )
                    nc.vector.tensor_add(
                        out=ot[:, ns, :], in0=o_ps, in1=x_q[ch][:, ns, :]
                    )
                    nc.gpsimd.tensor_add(
                        out=ot[:, ns, :], in0=ot[:, ns, :], in1=beff_sb
                    )
                nc.sync.dma_start(
                    out=out_r[:, ch * NS : (ch + 1) * NS, :], in_=ot
                )

    nc.finalize()
    return nc


def _get_nc():
    if "nc" not in _CACHE:
        _CACHE["nc"] = _build_nc()
    return _CACHE["nc"]


def _prep_in_maps(x, gamma, beta, wq, bq, wk, bk, wv, bv, wo, bo):
    f32 = np.float32
    x = np.asarray(x, f32)
    gamma = np.asarray(gamma, f32)
    beta = np.asarray(beta, f32)
    wq = np.asarray(wq, f32)
    bq = np.asarray(bq, f32)
    wk = np.asarray(wk, f32)
    bk = np.asarray(bk, f32)
    wv = np.asarray(wv, f32)
    bv = np.asarray(bv, f32)
    wo = np.asarray(wo, f32)
    bo = np.asarray(bo, f32)

    scale = 1.0 / np.sqrt(C).astype(f32)
    wq_e = gamma[:, None] * wq * scale
    bq_e = (beta @ wq + bq) * scale
    wk_e = gamma[:, None] * wk
    bk_e = beta @ wk + bk
    wv_e = gamma[:, None] * wv
    bv_e = beta @ wv + bv
    beff = bo + bv_e @ wo  # attention rows sum to 1 -> bv passes through

    def wl(w):  # [C, C] -> [P, KC, C] bf16 (cin on partitions)
        return np.ascontiguousarray(
            w.reshape(KC, P, C).transpose(1, 0, 2)
        ).astype(ml_dtypes.bfloat16)

    def bl(b):  # [C] -> [P, KC] f32 (cout on partitions)
        return np.ascontiguousarray(b.reshape(KC, P).T).astype(f32)

    shared = {
        "wq": wl(wq_e),
        "wk": wl(wk_e),
        "wv": wl(wv_e),
        "wo": wl(wo),
        "bq": bl(bq_e),
        "bk": bl(bk_e),
        "beff": np.ascontiguousarray(np.tile(beff[None, :], (P, 1))).astype(f32),
    }
    in_maps = []
    for b in range(B):
        m = dict(shared)
        m["x"] = np.ascontiguousarray(x[b].reshape(N, C)).astype(f32)
        in_maps.append(m)
    return in_maps


def run(trace=False, **inputs):
    """Build+run; returns (output, BassKernelResults)."""
    from concourse.bass_utils import run_bass_kernel_spmd

    in_maps = _prep_in_maps(**inputs)
    nc = _get_nc()
    res = run_bass_kernel_spmd(nc, in_maps, core_ids=list(range(B)), trace=trace)
    out = np.stack([r["out"] for r in res.results]).reshape(B, H, W, C)
    return out, res


def kernel(**inputs):
    out, _ = run(trace=False, **inputs)
    return out


# revision 30
# speedup vs baseline: 1.2052x; 1.2052x over previous
"""Trainium2 Bass kernel for nn_AttentionBlock (LN -> single-head attention -> residual).

Sharding: pure data parallelism — batch B=8, one batch element per NeuronCore.
Per core (N=4096 tokens, C=256 channels):
  LN (token-partition layout) -> PE transpose to hT [C, N] -> QT/KT/V projections
  -> per-1024-query-chunk: scoresT (keys on partitions, fp8 DoubleRow K=256)
     -> exp (ACT, fp8 out, exp(s - ln4) to fit e4m3 range) -> PV + rowsum
     matmuls (fp8 DoubleRow over key pairs) -> normalize
  -> output projection + residual (final phase).
Projections/transpose in bf16; attention matmuls in fp8e4m3 with DoubleRow
(2 fp8/cell, K=256 per matmul); all accumulation in fp32 PSUM except the
single-shot scores matmul (bf16 PSUM). The exp/rowsum scale factor 1/4
cancels in the softmax normalization.
gamma/beta/attention-scale/biases are folded into the weights on the host:
  wq' = gamma*wq/sqrt(C), bq' = (beta@wq + bq)/sqrt(C), wk' = gamma*wk,
  bk' = beta@wk + bk, wv' = gamma*wv, beff = bo + (beta@wv + bv)@wo.
"""

import math
import os
import sys

for _p in ("/opt/trn_rl_repo", "/opt/pypackages"):
    if os.path.isdir(_p) and _p not in sys.path:
        sys.path.append(_p)

import ml_dtypes
import numpy as np

B, H, W, C = 8, 64, 64, 256
N = H * W          # 4096 tokens per batch element
P = 128            # partitions
NT = N // P        # 32 token tiles / key blocks
KC = C // P        # 2 channel sub-tiles
CHUNK = 1024       # query-chunk size
NCH = N // CHUNK   # 4 chunks
HB = 512           # psum half-bank split for fp32 accumulators
LN_EPS = 1e-3
LN4 = math.log(4.0)

_CACHE = {}


def _build_nc():
    import concourse.mybir as mybir
    import concourse.tile as tile
    from concourse.bacc import Bacc
    from concourse.masks import make_identity

    f32 = mybir.dt.float32
    bf16 = mybir.dt.bfloat16
    f8 = mybir.dt.float8e4
    AF = mybir.ActivationFunctionType
    Alu = mybir.AluOpType
    DR = mybir.MatmulPerfMode.DoubleRow

    nc = Bacc()

    x_d = nc.declare_dram_parameter("x", [N, C], f32, isOutput=False)
    wq_d = nc.declare_dram_parameter("wq", [P, KC, C], bf16, isOutput=False)
    wk_d = nc.declare_dram_parameter("wk", [P, KC, C], bf16, isOutput=False)
    wv_d = nc.declare_dram_parameter("wv", [P, KC, C], bf16, isOutput=False)
    wo_d = nc.declare_dram_parameter("wo", [P, KC, C], bf16, isOutput=False)
    bq_d = nc.declare_dram_parameter("bq", [P, KC], f32, isOutput=False)
    bk_d = nc.declare_dram_parameter("bk", [P, KC], f32, isOutput=False)
    beff_d = nc.declare_dram_parameter("beff", [P, C], f32, isOutput=False)
    out_d = nc.declare_dram_parameter("out", [N, C], f32, isOutput=True)

    x_r = x_d[:].rearrange("(t p) c -> p t c", p=P)
    out_r = out_d[:].rearrange("(t p) c -> p t c", p=P)

    with tile.TileContext(nc) as tc:
        with (
            tc.tile_pool(name="consts", bufs=1) as consts,
            tc.tile_pool(name="big", bufs=1) as big,
            tc.tile_pool(name="lnp", bufs=4) as lnp,
            tc.tile_pool(name="expp", bufs=20) as expp,
            tc.tile_pool(name="rsbp", bufs=2) as rsbp,
            tc.tile_pool(name="outp", bufs=2) as outp,
            tc.tile_pool(name="ps_s", bufs=2, space="PSUM") as ps_s,
            tc.tile_pool(name="ps_pv", bufs=1, space="PSUM") as ps_pv,
            tc.tile_pool(name="ps_rs", bufs=1, space="PSUM") as ps_rs,
        ):
            # ---- constants / weights ----
            wq_sb = consts.tile([P, KC, C], bf16, tag="wq")
            wk_sb = consts.tile([P, KC, C], bf16, tag="wk")
            wv_sb = consts.tile([P, KC, C], bf16, tag="wv")
            wo_sb = consts.tile([P, KC, C], bf16, tag="wo")
            bq_sb = consts.tile([P, KC], f32, tag="bq")
            bk_sb = consts.tile([P, KC], f32, tag="bk")
            beff_sb = consts.tile([P, C], f32, tag="beff")
            nc.sync.dma_start(out=wq_sb, in_=wq_d[:])
            nc.sync.dma_start(out=wk_sb, in_=wk_d[:])
            nc.sync.dma_start(out=wv_sb, in_=wv_d[:])
            nc.sync.dma_start(out=wo_sb, in_=wo_d[:])
            nc.sync.dma_start(out=bq_sb, in_=bq_d[:])
            nc.sync.dma_start(out=bk_sb, in_=bk_d[:])
            nc.sync.dma_start(out=beff_sb, in_=beff_d[:])

            ident = consts.tile([P, P], bf16, tag="ident")
            make_identity(nc, ident)
            ones_sb = consts.tile([P, KC, P], f8, tag="ones")
            nc.vector.memset(ones_sb, 1.0)
            eps_sb = consts.tile([P, 1], f32, tag="eps")
            nc.vector.memset(eps_sb, LN_EPS)
            nln4_sb = consts.tile([P, 1], f32, tag="nln4")
            nc.vector.memset(nln4_sb, -LN4)

            # ---- load x (quartered for pipelining) ----
            NTQ = NT // NCH  # token tiles per quarter (8)
            x_q = [
                big.tile([P, NTQ, C], f32, tag=f"x{q}", name=f"x_q{q}")
                for q in range(NCH)
            ]
            for q in range(NCH):
                nc.sync.dma_start(
                    out=x_q[q], in_=x_r[:, q * NTQ : (q + 1) * NTQ, :]
                )

            # ---- LayerNorm (no affine; folded into weights) ----
            # stats on DVE, normalize applied on ACT: (x - mean)*rstd =
            # Identity(x*rstd + (-mean*rstd))
            xn = big.tile([P, NT, C], bf16, tag="xn")
            for t in range(NT):
                x_t = x_q[t // NTQ][:, t % NTQ, :]
                stats = lnp.tile([P, 6], f32, tag="stats")
                nc.vector.bn_stats(out=stats, in_=x_t)
                mv = lnp.tile([P, 2], f32, tag="mv")
                nc.vector.bn_aggr(out=mv, in_=stats)
                std = lnp.tile([P, 1], f32, tag="std")
                nc.scalar.activation(
                    out=std, in_=mv[:, 1:2], func=AF.Sqrt, bias=eps_sb
                )
                rstd = lnp.tile([P, 1], f32, tag="rstd")
                nc.vector.reciprocal(out=rstd, in_=std)
                nc.vector.tensor_scalar(
                    out=xn[:, t, :],
                    in0=x_t,
                    scalar1=mv[:, 0:1],
                    scalar2=rstd,
                    op0=Alu.subtract,
                    op1=Alu.mult,
                )

            # ---- transpose xn -> hT [C(part), N(free)], quartered ----
            hT_q = [
                big.tile([P, KC, CHUNK], bf16, tag=f"hT{q}", name=f"hT_q{q}")
                for q in range(NCH)
            ]
            for cj in range(KC):
                for tq in range(NT // 4):
                    t_ps = ps_s.tile([P, 512], bf16, tag="s")
                    for j in range(4):
                        ti = tq * 4 + j
                        nc.tensor.transpose(
                            t_ps[:, j * P : (j + 1) * P],
                            xn[:, ti, cj * P : (cj + 1) * P],
                            ident,
                        )
                    q = tq // 2
                    off = (tq % 2) * 512
                    nc.scalar.copy(
                        out=hT_q[q][:, cj, off : off + 512], in_=t_ps
                    )

            # ---- Q/K projections -> qT, kT [Cqk(part), N(free)], fp8 ----
            qT = big.tile([P, KC, N], f8, tag="qT")
            kT = big.tile([P, KC, N], f8, tag="kT")
            for w_sb, b_sb, dst in ((wq_sb, bq_sb, qT), (wk_sb, bk_sb, kT)):
                for c2 in range(KC):
                    for nch in range(NCH):
                        ps = ps_s.tile([P, CHUNK], f32, tag="s")
                        for hb in range(CHUNK // HB):
                            off = hb * HB
                            for kc in range(KC):
                                nc.tensor.matmul(
                                    ps[:, hb * HB : (hb + 1) * HB],
                                    lhsT=w_sb[:, kc, c2 * P : (c2 + 1) * P],
                                    rhs=hT_q[nch][:, kc, off : off + HB],
                                    start=(kc == 0),
                                    stop=(kc == KC - 1),
                                )
                        nc.scalar.activation(
                            out=dst[:, c2, nch * CHUNK : (nch + 1) * CHUNK],
                            in_=ps,
                            func=AF.Identity,
                            bias=b_sb[:, c2 : c2 + 1],
                        )

            # ---- V projection -> v [m(part), Cv(free)], fp8 ----
            v_sb = big.tile([P, NT, C], f8, tag="v")
            for mb in range(NT):
                ps = ps_s.tile([P, C], f32, tag="s")
                q, off = mb // NTQ, (mb % NTQ) * P
                for kc in range(KC):
                    nc.tensor.matmul(
                        ps,
                        lhsT=hT_q[q][:, kc, off : off + P],
                        rhs=wv_sb[:, kc, :],
                        start=(kc == 0),
                        stop=(kc == KC - 1),
                    )
                nc.scalar.copy(out=v_sb[:, mb, :], in_=ps)

            # ---- attention, one 1024-query chunk at a time ----
            # normalized attention output (transposed layout) for ALL chunks;
            # reuses xn's SBUF slot (xn's last reader is the transpose phase)
            attnT = big.tile([P, KC, N], bf16, tag="xn")
            NPAIR = NT // 2
            for ch in range(NCH):
                q_sl = slice(ch * CHUNK, (ch + 1) * CHUNK)
                rs_ps = ps_rs.tile([P, CHUNK], f32, tag="rs")
                es = []
                # phase a: scores (fp8 DR, K=256 one-shot) -> exp -> fp8 pairs,
                # with the ones-matmul row-sum riding along
                for pr in range(NPAIR):
                    e = expp.tile([P, 2, CHUNK], f8, tag="e")
                    es.append(e)
                    for j in range(2):
                        mb = 2 * pr + j
                        s_ps = ps_s.tile([P, CHUNK], f32, tag="s")
                        for hb in range(CHUNK // HB):
                            nc.tensor.matmul(
                                s_ps[:, hb * HB : (hb + 1) * HB],
                                lhsT=kT[:, 0:KC, mb * P : (mb + 1) * P],
                                rhs=qT[:, 0:KC, ch * CHUNK + hb * HB : ch * CHUNK + (hb + 1) * HB],
                                start=True,
                                stop=True,
                                perf_mode=DR,
                            )
                        # exp(s - ln4): scale by 1/4 so e4m3 max (448) is safe;
                        # the factor cancels between PV and the row-sum.
                        nc.scalar.activation(
                            out=e[:, j, :], in_=s_ps, func=AF.Exp, bias=nln4_sb
                        )
                    for hb in range(CHUNK // HB):
                        nc.tensor.matmul(
                            rs_ps[:, hb * HB : (hb + 1) * HB],
                            lhsT=ones_sb,
                            rhs=e[:, 0:2, hb * HB : (hb + 1) * HB],
                            start=(pr == 0),
                            stop=(pr == NPAIR - 1),
                            perf_mode=DR,
                            skip_group_check=True,
                        )
                # reciprocal of row-sums (replicated across partitions)
                rs_sb = rsbp.tile([P, CHUNK], f32, tag="rssb")
                nc.vector.reciprocal(out=rs_sb, in_=rs_ps)
                # phase b: PV (fp8 DR over key pairs), one c-tile at a time
                for c2 in range(KC):
                    pv = ps_pv.tile([P, CHUNK], f32, tag="pv")
                    for pr in range(NPAIR):
                        for hb in range(CHUNK // HB):
                            nc.tensor.matmul(
                                pv[:, hb * HB : (hb + 1) * HB],
                                lhsT=v_sb[:, 2 * pr : 2 * pr + 2, c2 * P : (c2 + 1) * P],
                                rhs=es[pr][:, 0:2, hb * HB : (hb + 1) * HB],
                                start=(pr == 0),
                                stop=(pr == NPAIR - 1),
                                perf_mode=DR,
                                skip_group_check=True,
                            )
                    nc.vector.tensor_mul(
                        out=attnT[:, c2, q_sl], in0=pv, in1=rs_sb
                    )

                # ---- output projection + residual + bias for this chunk ----
                NS = CHUNK // P
                ot = outp.tile([P, NS, C], f32, tag="out")
                for ns in range(NS):
                    o_ps = ps_pv.tile([P, C], f32, tag="pv")
                    for kc in range(KC):
                        nc.tensor.matmul(
                            o_ps,
                            lhsT=attnT[
                                :, kc, ch * CHUNK + ns * P : ch * CHUNK + (ns + 1) * P
                            ],
                            rhs=wo_sb[:, kc, :],
                            start=(kc == 0),
                            stop=(kc == KC - 1),
                        )
                    nc.vector.tensor_add(
                        out=ot[:, ns, :], in0=o_ps, in1=x_q[ch][:, ns, :]
                    )
                    nc.gpsimd.tensor_add(
                        out=ot[:, ns, :], in0=ot[:, ns, :], in1=beff_sb
                    )
                nc.sync.dma_start(
                    out=out_r[:, ch * NS : (ch + 1) * NS, :], in_=ot
                )

    nc.finalize()
    return nc


def _get_nc():
    if "nc" not in _CACHE:
        _CACHE["nc"] = _build_nc()
    return _CACHE["nc"]


def _prep_in_maps(x, gamma, beta, wq, bq, wk, bk, wv, bv, wo, bo):
    f32 = np.float32
    x = np.asarray(x, f32)
    gamma = np.asarray(gamma, f32)
    beta = np.asarray(beta, f32)
    wq = np.asarray(wq, f32)
    bq = np.asarray(bq, f32)
    wk = np.asarray(wk, f32)
    bk = np.asarray(bk, f32)
    wv = np.asarray(wv, f32)
    bv = np.asarray(bv, f32)
    wo = np.asarray(wo, f32)
    bo = np.asarray(bo, f32)

    scale = 1.0 / np.sqrt(C).astype(f32)
    wq_e = gamma[:, None] * wq * scale
    bq_e = (beta @ wq + bq) * scale
    wk_e = gamma[:, None] * wk
    bk_e = beta @ wk + bk
    wv_e = gamma[:, None] * wv
    bv_e = beta @ wv + bv
    beff = bo + bv_e @ wo  # attention rows sum to 1 -> bv passes through

    def wl(w):  # [C, C] -> [P, KC, C] bf16 (cin on partitions)
        return np.ascontiguousarray(
            w.reshape(KC, P, C).transpose(1, 0, 2)
        ).astype(ml_dtypes.bfloat16)

    def bl(b):  # [C] -> [P, KC] f32 (cout on partitions)
        return np.ascontiguousarray(b.reshape(KC, P).T).astype(f32)

    shared = {
        "wq": wl(wq_e),
        "wk": wl(wk_e),
        "wv": wl(wv_e),
        "wo": wl(wo),
        "bq": bl(bq_e),
        "bk": bl(bk_e),
        "beff": np.ascontiguousarray(np.tile(beff[None, :], (P, 1))).astype(f32),
    }
    in_maps = []
    for b in range(B):
        m = dict(shared)
        m["x"] = np.ascontiguousarray(x[b].reshape(N, C)).astype(f32)
        in_maps.append(m)
    return in_maps


def run(trace=False, **inputs):
    """Build+run; returns (output, BassKernelResults)."""
    from concourse.bass_utils import run_bass_kernel_spmd

    in_maps = _prep_in_maps(**inputs)
    nc = _get_nc()
    res = run_bass_kernel_spmd(nc, in_maps, core_ids=list(range(B)), trace=trace)
    out = np.stack([r["out"] for r in res.results]).reshape(B, H, W, C)
    return out, res


def kernel(**inputs):
    out, _ = run(trace=False, **inputs)
    return out


# revision 38
# speedup vs baseline: 1.3247x; 1.0992x over previous
"""Trainium2 Bass kernel for nn_AttentionBlock (LN -> single-head attention -> residual).

Sharding: pure data parallelism — batch B=8, one batch element per NeuronCore.
Per core (N=4096 tokens, C=256 channels):
  LN (token-partition layout) -> PE transpose to hT [C, N] -> QT/KT/V projections
  -> per-1024-query-chunk: scoresT (keys on partitions, fp8 DoubleRow K=256)
     -> exp (ACT, fp8 out, exp(s - ln4) to fit e4m3 range) -> PV + rowsum
     matmuls (fp8 DoubleRow over key pairs) -> normalize
  -> output projection + residual (final phase).
Projections/transpose in bf16; attention matmuls in fp8e4m3 with DoubleRow
(2 fp8/cell, K=256 per matmul); all accumulation in fp32 PSUM except the
single-shot scores matmul (bf16 PSUM). The exp/rowsum scale factor 1/4
cancels in the softmax normalization.
gamma/beta/attention-scale/biases are folded into the weights on the host:
  wq' = gamma*wq/sqrt(C), bq' = (beta@wq + bq)/sqrt(C), wk' = gamma*wk,
  bk' = beta@wk + bk, wv' = gamma*wv, beff = bo + (beta@wv + bv)@wo.
"""

import math
import os
import sys

for _p in ("/opt/trn_rl_repo", "/opt/pypackages"):
    if os.path.isdir(_p) and _p not in sys.path:
        sys.path.append(_p)

import ml_dtypes
import numpy as np

B, H, W, C = 8, 64, 64, 256
N = H * W          # 4096 tokens per batch element
P = 128            # partitions
NT = N // P        # 32 token tiles / key blocks
KC = C // P        # 2 channel sub-tiles
CHUNK = 1024       # query-chunk size
NCH = N // CHUNK   # 4 chunks
HB = 512           # psum half-bank split for fp32 accumulators
LN_EPS = 1e-3
LN4 = math.log(4.0)

_CACHE = {}


def _build_nc():
    import concourse.mybir as mybir
    import concourse.tile as tile
    from concourse.bacc import Bacc
    from concourse.masks import make_identity

    f32 = mybir.dt.float32
    bf16 = mybir.dt.bfloat16
    f8 = mybir.dt.float8e4
    AF = mybir.ActivationFunctionType
    Alu = mybir.AluOpType
    DR = mybir.MatmulPerfMode.DoubleRow

    nc = Bacc()

    x_d = nc.declare_dram_parameter("x", [N, C], f32, isOutput=False)
    wq_d = nc.declare_dram_parameter("wq", [P, KC, C], bf16, isOutput=False)
    wk_d = nc.declare_dram_parameter("wk", [P, KC, C], bf16, isOutput=False)
    wv_d = nc.declare_dram_parameter("wv", [P, KC, C], bf16, isOutput=False)
    wo_d = nc.declare_dram_parameter("wo", [P, KC, C], bf16, isOutput=False)
    bq_d = nc.declare_dram_parameter("bq", [P, KC], f32, isOutput=False)
    bk_d = nc.declare_dram_parameter("bk", [P, KC], f32, isOutput=False)
    beff_d = nc.declare_dram_parameter("beff", [P, C], f32, isOutput=False)
    out_d = nc.declare_dram_parameter("out", [N, C], f32, isOutput=True)

    x_r = x_d[:].rearrange("(t p) c -> p t c", p=P)
    out_r = out_d[:].rearrange("(t p) c -> p t c", p=P)

    with tile.TileContext(nc) as tc:
        with (
            tc.tile_pool(name="consts", bufs=1) as consts,
            tc.tile_pool(name="big", bufs=1) as big,
            tc.tile_pool(name="lnp", bufs=4) as lnp,
            tc.tile_pool(name="expp", bufs=32) as expp,
            tc.tile_pool(name="rsbp", bufs=2) as rsbp,
            tc.tile_pool(name="outp", bufs=2) as outp,
            tc.tile_pool(name="ps_s", bufs=2, space="PSUM") as ps_s,
            tc.tile_pool(name="ps_pv", bufs=2, space="PSUM") as ps_pv,
            tc.tile_pool(name="ps_rs", bufs=1, space="PSUM") as ps_rs,
        ):
            # ---- constants / weights ----
            wq_sb = consts.tile([P, KC, C], bf16, tag="wq")
            wk_sb = consts.tile([P, KC, C], bf16, tag="wk")
            wv_sb = consts.tile([P, KC, C], bf16, tag="wv")
            wo_sb = consts.tile([P, KC, C], bf16, tag="wo")
            bq_sb = consts.tile([P, KC], f32, tag="bq")
            bk_sb = consts.tile([P, KC], f32, tag="bk")
            beff_sb = consts.tile([P, C], f32, tag="beff")
            nc.sync.dma_start(out=wq_sb, in_=wq_d[:])
            nc.sync.dma_start(out=wk_sb, in_=wk_d[:])
            nc.sync.dma_start(out=wv_sb, in_=wv_d[:])
            nc.sync.dma_start(out=wo_sb, in_=wo_d[:])
            nc.sync.dma_start(out=bq_sb, in_=bq_d[:])
            nc.sync.dma_start(out=bk_sb, in_=bk_d[:])
            nc.sync.dma_start(out=beff_sb, in_=beff_d[:])

            ident = consts.tile([P, P], bf16, tag="ident")
            make_identity(nc, ident)
            ones_sb = consts.tile([P, KC, P], f8, tag="ones")
            nc.vector.memset(ones_sb, 1.0)
            eps_sb = consts.tile([P, 1], f32, tag="eps")
            nc.vector.memset(eps_sb, LN_EPS)
            nln4_sb = consts.tile([P, 1], f32, tag="nln4")
            nc.vector.memset(nln4_sb, -LN4)

            # ---- load x (quartered for pipelining) ----
            NTQ = NT // NCH  # token tiles per quarter (8)
            x_q = [
                big.tile([P, NTQ, C], f32, tag=f"x{q}", name=f"x_q{q}")
                for q in range(NCH)
            ]
            for q in range(NCH):
                nc.sync.dma_start(
                    out=x_q[q], in_=x_r[:, q * NTQ : (q + 1) * NTQ, :]
                )

            # ---- LayerNorm (no affine; folded into weights) ----
            # stats on DVE, normalize applied on ACT: (x - mean)*rstd =
            # Identity(x*rstd + (-mean*rstd))
            xn = big.tile([P, NT, C], bf16, tag="xn")
            for t in range(NT):
                x_t = x_q[t // NTQ][:, t % NTQ, :]
                stats = lnp.tile([P, 6], f32, tag="stats")
                nc.vector.bn_stats(out=stats, in_=x_t)
                mv = lnp.tile([P, 2], f32, tag="mv")
                nc.vector.bn_aggr(out=mv, in_=stats)
                std = lnp.tile([P, 1], f32, tag="std")
                nc.scalar.activation(
                    out=std, in_=mv[:, 1:2], func=AF.Sqrt, bias=eps_sb
                )
                rstd = lnp.tile([P, 1], f32, tag="rstd")
                nc.vector.reciprocal(out=rstd, in_=std)
                nc.vector.tensor_scalar(
                    out=xn[:, t, :],
                    in0=x_t,
                    scalar1=mv[:, 0:1],
                    scalar2=rstd,
                    op0=Alu.subtract,
                    op1=Alu.mult,
                )

            # ---- transpose xn -> hT [C(part), N(free)], quartered ----
            hT_q = [
                big.tile([P, KC, CHUNK], bf16, tag=f"hT{q}", name=f"hT_q{q}")
                for q in range(NCH)
            ]
            for q in range(NCH):
                for tq in (2 * q, 2 * q + 1):
                    for cj in range(KC):
                        t_ps = ps_s.tile([P, 512], bf16, tag="s", name="t_ps")
                        for j in range(4):
                            ti = tq * 4 + j
                            nc.tensor.transpose(
                                t_ps[:, j * P : (j + 1) * P],
                                xn[:, ti, cj * P : (cj + 1) * P],
                                ident,
                            )
                        off = (tq % 2) * 512
                        nc.scalar.copy(
                            out=hT_q[q][:, cj, off : off + 512], in_=t_ps
                        )

            # ---- Q/K/V projections, quarter-major so attention can start
            # before all projections finish. qT/kT [Cqk(part), N(free)] fp8;
            # v [m(part), Cv(free)] fp8 ----
            qT_q = [
                big.tile([P, KC, CHUNK], f8, tag=f"qT{q}", name=f"qT_q{q}")
                for q in range(NCH)
            ]
            kT_q = [
                big.tile([P, KC, CHUNK], f8, tag=f"kT{q}", name=f"kT_q{q}")
                for q in range(NCH)
            ]
            v_q = [
                big.tile([P, NTQ, C], f8, tag=f"v{q}", name=f"v_q{q}")
                for q in range(NCH)
            ]
            for q in range(NCH):
                for w_sb, b_sb, dst in (
                    (wq_sb, bq_sb, qT_q[q]),
                    (wk_sb, bk_sb, kT_q[q]),
                ):
                    for c2 in range(KC):
                        for hb in range(CHUNK // HB):
                            ps = ps_pv.tile([P, HB], f32, tag="pv", name="qk_ps")
                            for kc in range(KC):
                                nc.tensor.matmul(
                                    ps,
                                    lhsT=w_sb[:, kc, c2 * P : (c2 + 1) * P],
                                    rhs=hT_q[q][:, kc, hb * HB : (hb + 1) * HB],
                                    start=(kc == 0),
                                    stop=(kc == KC - 1),
                                )
                            nc.scalar.activation(
                                out=dst[:, c2, hb * HB : (hb + 1) * HB],
                                in_=ps,
                                func=AF.Identity,
                                bias=b_sb[:, c2 : c2 + 1],
                            )
                for mbq in range(NTQ):
                    ps = ps_pv.tile([P, C], f32, tag="pv", name="v_ps")
                    # noqa: fits the [P, HB] pv slot (C == 256 <= HB)
                    for kc in range(KC):
                        nc.tensor.matmul(
                            ps,
                            lhsT=hT_q[q][:, kc, mbq * P : (mbq + 1) * P],
                            rhs=wv_sb[:, kc, :],
                            start=(kc == 0),
                            stop=(kc == KC - 1),
                        )
                    nc.scalar.copy(out=v_q[q][:, mbq, :], in_=ps)

            # ---- attention: chunks software-pipelined at key-pair depth.
            # Step st runs scores/exp/rowsum of chunk st interleaved with the
            # PV matmuls of chunk st-1, so ACT (exp) and PE stay busy
            # together. All attention matmuls are fp8 DoubleRow (K=256).
            # normalized attention output (transposed layout) for ALL chunks;
            # reuses xn's SBUF slot (xn's last reader is the transpose phase)
            attnT = big.tile([P, KC, N], bf16, tag="xn")
            NPAIR = NT // 2
            NS = CHUNK // P
            NHB = CHUNK // HB

            def rs_mm(rs_ps, e, first, last):
                for hb in range(NHB):
                    nc.tensor.matmul(
                        rs_ps[:, hb * HB : (hb + 1) * HB],
                        lhsT=ones_sb,
                        rhs=e[:, 0:2, hb * HB : (hb + 1) * HB],
                        start=first,
                        stop=last,
                        perf_mode=DR,
                        skip_group_check=True,
                    )

            e_tiles = {}
            rssb_tiles = {}
            for st in range(NCH + 1):
                pch = st - 1
                if st < NCH:
                    rs_ps = ps_rs.tile([P, CHUNK], f32, tag="rs", name="rs_ps")
                pv_nh = None
                for pr in range(NPAIR):
                    # PV of the previous chunk: half nh = pr//8 sweeps ALL 16
                    # key pairs (2 per scores-pair) for one 512-wide n-half,
                    # so only 2 PSUM banks of accumulator are live at a time.
                    if st > 0:
                        nh = pr // (NPAIR // 2)
                        if pr % (NPAIR // 2) == 0:
                            pv_nh = (
                                ps_pv.tile([P, HB], f32, tag="pv", name="pv0"),
                                ps_pv.tile([P, HB], f32, tag="pv", name="pv1"),
                            )
                        for k in (2 * (pr % 8), 2 * (pr % 8) + 1):
                            e_prev = e_tiles[(pch, k)]
                            vq = v_q[(2 * k) // NTQ]
                            voff = (2 * k) % NTQ
                            for c2 in range(KC):
                                nc.tensor.matmul(
                                    pv_nh[c2],
                                    lhsT=vq[
                                        :, voff : voff + 2, c2 * P : (c2 + 1) * P
                                    ],
                                    rhs=e_prev[:, 0:2, nh * HB : (nh + 1) * HB],
                                    start=(k == 0),
                                    stop=(k == NPAIR - 1),
                                    perf_mode=DR,
                                    skip_group_check=True,
                                )
                        if pr % (NPAIR // 2) == (NPAIR // 2) - 1:
                            # half done: normalize into attnT, freeing psum
                            for c2 in range(KC):
                                nc.vector.tensor_mul(
                                    out=attnT[
                                        :,
                                        c2,
                                        pch * CHUNK + nh * HB : pch * CHUNK + (nh + 1) * HB,
                                    ],
                                    in0=pv_nh[c2],
                                    in1=rssb_tiles[pch][:, nh * HB : (nh + 1) * HB],
                                )
                    if st < NCH:
                        e = expp.tile([P, 2, CHUNK], f8, tag="e", name="e")
                        e_tiles[(st, pr)] = e
                        for j in range(2):
                            mb = 2 * pr + j
                            s_ps = ps_s.tile([P, CHUNK], f32, tag="s", name="s_ps")
                            for hb in range(NHB):
                                nc.tensor.matmul(
                                    s_ps[:, hb * HB : (hb + 1) * HB],
                                    lhsT=kT_q[mb // NTQ][
                                        :, 0:KC, (mb % NTQ) * P : (mb % NTQ + 1) * P
                                    ],
                                    rhs=qT_q[st][:, 0:KC, hb * HB : (hb + 1) * HB],
                                    start=True,
                                    stop=True,
                                    perf_mode=DR,
                                )
                            # exp(s - ln4): scale by 1/4 so e4m3 max (448) is
                            # safe; the factor cancels in the normalization.
                            nc.scalar.activation(
                                out=e[:, j, :], in_=s_ps, func=AF.Exp,
                                bias=nln4_sb,
                            )
                        if pr > 0:
                            rs_mm(rs_ps, e_tiles[(st, pr - 1)], pr == 1, False)
                if st < NCH:
                    rs_mm(rs_ps, e_tiles[(st, NPAIR - 1)], False, True)
                    # reciprocal of row-sums (replicated across partitions)
                    rs_sb = rsbp.tile([P, CHUNK], f32, tag="rssb", name="rs_sb")
                    nc.vector.reciprocal(out=rs_sb, in_=rs_ps)
                    rssb_tiles[st] = rs_sb
                if st > 0:
                    # output projection + residual + bias for chunk pch
                    ot = outp.tile([P, NS, C], f32, tag="out", name="ot")
                    for ns in range(NS):
                        o_ps = ps_pv.tile([P, C], f32, tag="pv", name="o_ps")
                        for kc in range(KC):
                            nc.tensor.matmul(
                                o_ps,
                                lhsT=attnT[
                                    :,
                                    kc,
                                    pch * CHUNK + ns * P : pch * CHUNK + (ns + 1) * P,
                                ],
                                rhs=wo_sb[:, kc, :],
                                start=(kc == 0),
                                stop=(kc == KC - 1),
                            )
                        nc.vector.tensor_add(
                            out=ot[:, ns, :], in0=o_ps, in1=x_q[pch][:, ns, :]
                        )
                        nc.gpsimd.tensor_add(
                            out=ot[:, ns, :], in0=ot[:, ns, :], in1=beff_sb
                        )
                    nc.sync.dma_start(
                        out=out_r[:, pch * NS : (pch + 1) * NS, :], in_=ot
                    )

    nc.finalize()
    return nc


def _get_nc():
    if "nc" not in _CACHE:
        _CACHE["nc"] = _build_nc()
    return _CACHE["nc"]


def _prep_in_maps(x, gamma, beta, wq, bq, wk, bk, wv, bv, wo, bo):
    f32 = np.float32
    x = np.asarray(x, f32)
    gamma = np.asarray(gamma, f32)
    beta = np.asarray(beta, f32)
    wq = np.asarray(wq, f32)
    bq = np.asarray(bq, f32)
    wk = np.asarray(wk, f32)
    bk = np.asarray(bk, f32)
    wv = np.asarray(wv, f32)
    bv = np.asarray(bv, f32)
    wo = np.asarray(wo, f32)
    bo = np.asarray(bo, f32)

    scale = 1.0 / np.sqrt(C).astype(f32)
    wq_e = gamma[:, None] * wq * scale
    bq_e = (beta @ wq + bq) * scale
    wk_e = gamma[:, None] * wk
    bk_e = beta @ wk + bk
    wv_e = gamma[:, None] * wv
    bv_e = beta @ wv + bv
    beff = bo + bv_e @ wo  # attention rows sum to 1 -> bv passes through

    def wl(w):  # [C, C] -> [P, KC, C] bf16 (cin on partitions)
        return np.ascontiguousarray(
            w.reshape(KC, P, C).transpose(1, 0, 2)
        ).astype(ml_dtypes.bfloat16)

    def bl(b):  # [C] -> [P, KC] f32 (cout on partitions)
        return np.ascontiguousarray(b.reshape(KC, P).T).astype(f32)

    shared = {
        "wq": wl(wq_e),
        "wk": wl(wk_e),
        "wv": wl(wv_e),
        "wo": wl(wo),
        "bq": bl(bq_e),
        "bk": bl(bk_e),
        "beff": np.ascontiguousarray(np.tile(beff[None, :], (P, 1))).astype(f32),
    }
    in_maps = []
    for b in range(B):
        m = dict(shared)
        m["x"] = np.ascontiguousarray(x[b].reshape(N, C)).astype(f32)
        in_maps.append(m)
    return in_maps


def run(trace=False, **inputs):
    """Build+run; returns (output, BassKernelResults)."""
    from concourse.bass_utils import run_bass_kernel_spmd

    in_maps = _prep_in_maps(**inputs)
    nc = _get_nc()
    res = run_bass_kernel_spmd(nc, in_maps, core_ids=list(range(B)), trace=trace)
    out = np.stack([r["out"] for r in res.results]).reshape(B, H, W, C)
    return out, res


def kernel(**inputs):
    out, _ = run(trace=False, **inputs)
    return out
